# revision 57
# baseline (speedup 1.0000x reference)
"""Trainium2 Bass kernel: MultiHeadAttention + residual + LayerNorm.

Problem shapes (hardcoded):
  x: (2, 2048, 1024) f32, 16 heads x 64 head_dim, scale = 64**-0.5
  y = LayerNorm(x + MHA(x))

Sharding: token-parallel over 8 cores. Core c handles batch b=c//4 and
query tokens [512*(c%4), 512*(c%4+1)) of that batch. Each core receives
its batch's full token sequence ROTATED so that its own 512 query tokens
are rows 0..511 (attention is permutation-invariant over keys). No
cross-core collectives.

Design (fp8 v5):
- fp8e4m3 operands for all matmuls; f32 residual/LayerNorm path.
  Weights host-prescaled x32 (their +-1/32 range lands in fp8
  subnormals otherwise), Wo x16; descales fold into PSUM evacuations.
- QKV projections, attn@V, and out-proj run in DoubleRow perf mode
  (2 contraction tiles per PE instruction). HW ISA constraints honored:
  stationary k-tile pairs contiguous, stationary free dim a multiple of
  16, DR outputs at partition base 0. Moving-operand pairs may stride.
- V is built token-major via stationary=x^T (a second, token-tile-major
  fp8 copy of x) -- no PE transposes; one fused (psum/32 + bv) -> fp8
  evacuation per 512 tokens.
- attn@V stationary is [V | (1/16)x16] (M=80): softmax denominators
  ride in psum rows 64:80 of the same accumulation at 1/16 scale.
  Normalization is PE-free: copy rows 64:80 to SBUF, fan out 16->64
  partitions with one SBUF-SBUF DMA (each source row read 4x via a
  0-step dim), reciprocal_approx_fast at partition base 0 (the custom
  DVE op corrupts at nonzero partition offsets), multiply. head1
  partition-shifts via one SBUF-SBUF DMA.
- Software pipelining: Q^T j-slices and the next pair's K/V chunks are
  emitted inside the previous pair's attention chunk loop, keeping the
  scalar engine (exp, the bottleneck: ~110us of irreducible work) fed
  across pair boundaries. PE warmup matmuls ramp the clock p-state and
  preload the Exp table during the initial DMAs; the Sqrt table is
  prefetched before the LayerNorm tail.
- DMA ordering: packed biases first, first-use weight/x slivers next,
  bulk weights after, Phase-D-only tensors (Wo, residual x, LN row
  constants) last. bo+x is pre-added on the Pool engine mid-kernel.
- LayerNorm tail: fused (po/256 + (bo+x)) evacuation, bn_stats/bn_aggr,
  gamma/beta skipped when they are identity (checked on host). The
  final normalize is OUT-OF-PLACE on DVE per 512-col half (in-place
  DVE/Pool tensor ops run 10-20x slower), each half's writeback on its
  own DMA queue.

(A token-sharded K/V AllGather variant -- compute K/V only for own 512
tokens, gather pairs across the batch's 4 cores -- was built and was
correct (see kernel_ag_299us.py) but lost ~25us to serial collective
latency plus SBUF-write contention during attention; the collective-
free version here is faster on this stack.)
"""

import sys

sys.path.insert(0, "/opt/trn_rl_repo")

import numpy as np
import ml_dtypes

import concourse.bass as bass
import concourse.bacc as bacc
import concourse.mybir as mybir
import concourse.tile as tile
from concourse import bass_utils

# ---- problem constants ----
B = 2
S = 2048
D = 1024
H = 16
DH = 64
SCALE = DH ** -0.5
EPS = 1e-5

N_CORES = 8
CORES_PER_BATCH = N_CORES // B
TQ = S // CORES_PER_BATCH          # 512 query tokens per core
NT = S // 128                      # 16 key tiles of 128
ND = D // 128                      # 8 dim tiles of 128
NPAIR = H // 2                     # 8 head pairs
NTQ = TQ // 128                    # 4 query tiles
WSCALE = 32.0                      # host premultiplies W by this
NCHUNK = 4                         # attention chunks per pair
TPC = NT // NCHUNK                 # key tiles per chunk

F32 = mybir.dt.float32
F32R = mybir.dt.float32r
BF16 = mybir.dt.bfloat16
F8 = mybir.dt.float8e4
DR = mybir.MatmulPerfMode.DoubleRow
MULT = mybir.AluOpType.mult
ADD = mybir.AluOpType.add

NP_F8 = ml_dtypes.float8_e4m3
NP_BF16 = ml_dtypes.bfloat16


def _build_program(apply_gb=True):
    nc = bacc.Bacc("TRN2", target_bir_lowering=False, debug=False,
                   num_devices=N_CORES)

    # ---- DRAM I/O ----
    xbP8_d = nc.dram_tensor("xbP8", (128, NT, ND, 128), F8,
                            kind="ExternalInput").ap()
    xq_d = nc.dram_tensor("xq", (TQ, D), F32, kind="ExternalInput").ap()
    wq8_d = nc.dram_tensor("wq8", (128, ND, ND, 128), F8,
                           kind="ExternalInput").ap()
    wk8_d = nc.dram_tensor("wk8", (128, ND, ND, 128), F8,
                           kind="ExternalInput").ap()
    wv8_d = nc.dram_tensor("wv8", (128, ND, ND, 128), F8,
                           kind="ExternalInput").ap()
    wo_d = nc.dram_tensor("wo", (128, ND, D), F8, kind="ExternalInput").ap()
    b3_d = nc.dram_tensor("b3", (128, 3, ND), F32, kind="ExternalInput").ap()
    bo_d = nc.dram_tensor("bo", (D,), F32, kind="ExternalInput").ap()
    gamma_d = nc.dram_tensor("gamma", (D,), F32, kind="ExternalInput").ap()
    beta_d = nc.dram_tensor("beta", (D,), F32, kind="ExternalInput").ap()
    y_d = nc.dram_tensor("y", (TQ, D), F32, kind="ExternalOutput").ap()

    def bcast_rows(src_row_ap, nrows):
        return bass.AP(tensor=src_row_ap.tensor, offset=src_row_ap.offset,
                       ap=[[0, nrows]] + [list(d) for d in src_row_ap.ap[-1:]])

    with tile.TileContext(nc) as tc:
        from contextlib import ExitStack
        with ExitStack() as ctx:
            consts = ctx.enter_context(tc.tile_pool(name="consts", bufs=1))
            bigp = ctx.enter_context(tc.tile_pool(name="big", bufs=1))
            ktp = ctx.enter_context(tc.tile_pool(name="ktp", bufs=3))
            vaug = ctx.enter_context(tc.tile_pool(name="vaug", bufs=3))
            expp = ctx.enter_context(tc.tile_pool(name="expp", bufs=8))
            smallp = ctx.enter_context(tc.tile_pool(name="small", bufs=2))
            ybufp = ctx.enter_context(tc.tile_pool(name="ybuf", bufs=4))
            statp = ctx.enter_context(tc.tile_pool(name="statp", bufs=4))

            ps_sc = ctx.enter_context(
                tc.tile_pool(name="ps_sc", bufs=2, space="PSUM"))
            ps_acc = ctx.enter_context(
                tc.tile_pool(name="ps_acc", bufs=2, space="PSUM"))
            ps_pav = ctx.enter_context(
                tc.tile_pool(name="ps_pav", bufs=2, space="PSUM"))

            # ---- tiny loads first: biases (one packed DMA), constants ----
            b3_t = consts.tile([128, 3, ND], F32)
            nc.sync.dma_start(out=b3_t, in_=b3_d)
            bq_t, bk_t, bv_t = b3_t[:, 0, :], b3_t[:, 1, :], b3_t[:, 2, :]
            eps_t = consts.tile([128, 1], F32)
            nc.vector.memset(eps_t, EPS)
            c32 = consts.tile([128, 1], F32)
            nc.vector.memset(c32, 1.0 / WSCALE)
            ones_f = consts.tile([128, 64], F32)
            nc.vector.memset(ones_f, 1.0 / 16.0)  # folds x16 into attn-out
            ones_r = consts.tile([128, 64], F32R)
            nc.vector.tensor_copy(out=ones_r, in_=ones_f)

            # ---- warmup: ramp the PE clock + preload the Exp table while
            # the first DMAs are in flight (both are off the data path) ----
            warm = ps_acc.tile([128, TQ], F32, tag="acc", name="warm")
            for w in range(18):
                nc.tensor.matmul(warm[0:64, 0:64], ones_r[0:1, :],
                                 ones_r[0:1, :], start=True, stop=True)
            wsc = smallp.tile([128, 1], F32, tag="wsc", name="wsc")
            nc.scalar.activation(out=wsc, in_=eps_t,
                                 func=mybir.ActivationFunctionType.Exp,
                                 scale=1.0)

            # ---- compute-critical loads: Wq, x (per 512-token chunk);
            # the first-use slivers go out on four queues in parallel (the
            # first QKV emission is gated by these ~900KB) ----
            wq8 = bigp.tile([128, ND, ND, 128], F8, tag="wq8", name="wq8")
            nc.sync.dma_start(out=wq8[:, 0, :, :], in_=wq8_d[:, 0, :, :])
            xP8 = bigp.tile([128, NT, ND, 128], F8, tag="xP8", name="xP8")
            nc.scalar.dma_start(out=xP8[:, 0:2], in_=xbP8_d[:, 0:2])
            nc.sync.dma_start(out=xP8[:, 2:4], in_=xbP8_d[:, 2:4])
            wk8 = bigp.tile([128, ND, ND, 128], F8, tag="wk8", name="wk8")
            nc.gpsimd.dma_start(out=wk8[:, 0, :, :], in_=wk8_d[:, 0, :, :])
            wv8 = bigp.tile([128, ND, ND, 128], F8, tag="wv8", name="wv8")
            nc.gpsimd.dma_start(out=wv8[:, 0, :, :], in_=wv8_d[:, 0, :, :])
            nc.sync.dma_start(out=wq8[:, 1:2], in_=wq8_d[:, 1:2])
            nc.sync.dma_start(out=xP8[:, 4:8], in_=xbP8_d[:, 4:8])
            nc.sync.dma_start(out=wk8[:, 1:2], in_=wk8_d[:, 1:2])
            nc.sync.dma_start(out=wv8[:, 1:2], in_=wv8_d[:, 1:2])
            nc.sync.dma_start(out=xP8[:, 8:12], in_=xbP8_d[:, 8:12])
            nc.sync.dma_start(out=wq8[:, 2:3], in_=wq8_d[:, 2:3])
            nc.sync.dma_start(out=xP8[:, 12:16], in_=xbP8_d[:, 12:16])
            nc.sync.dma_start(out=wq8[:, 3:ND], in_=wq8_d[:, 3:ND])
            nc.sync.dma_start(out=wk8[:, 2:ND], in_=wk8_d[:, 2:ND])
            nc.sync.dma_start(out=wv8[:, 2:ND], in_=wv8_d[:, 2:ND])

            # ---- Phase-D-only loads: Wo, residual x, LN row consts ----
            wo_t = bigp.tile([128, ND, D], F8, tag="wo", name="wo")
            nc.sync.dma_start(out=wo_t, in_=wo_d)
            xq_t = bigp.tile([128, NTQ, D], F32, tag="xq", name="xq")
            nc.sync.dma_start(
                out=xq_t, in_=xq_d.rearrange("(i p) d -> p i d", p=128))
            lnc = consts.tile([128, 3, D], F32, name="lnc")
            nc.sync.dma_start(out=lnc[:, 0, :], in_=bcast_rows(bo_d[None], 128))
            nc.sync.dma_start(out=lnc[:, 1, :],
                              in_=bcast_rows(gamma_d[None], 128))
            nc.sync.dma_start(out=lnc[:, 2, :],
                              in_=bcast_rows(beta_d[None], 128))
            bo_b, gamma_b, beta_b = lnc[:, 0, :], lnc[:, 1, :], lnc[:, 2, :]

            # ---- emission helpers (software pipelining) ----
            qT8 = bigp.tile([128, ND, TQ], F8, tag="qT8", name="qT8")
            bxq = bigp.tile([128, NTQ, D], F32, tag="bxq", name="bxq")

            def xmov(dd, t):
                # [128, 2(d-pair), 128 tokens] moving view of xP8 tile t
                return xP8[:, t, 2 * dd:2 * dd + 2, :]

            def emit_q(j):
                # qT8[p, j, tq] = q[tq, 128j+p]; q = (psum/32) + bq
                pq = ps_acc.tile([128, TQ], F32, tag="acc", name=f"pq{j}")
                for s in range(4):
                    for dd in range(ND // 2):
                        nc.tensor.matmul(
                            pq[:, 128 * s:128 * (s + 1)],
                            wq8[:, j, 2 * dd:2 * dd + 2, :], xmov(dd, s),
                            start=(s == 0 and dd == 0),
                            stop=(s == 3 and dd == ND // 2 - 1),
                            perf_mode=DR, skip_group_check=True)
                nc.vector.tensor_scalar(
                    out=qT8[:, j, :], in0=pq, scalar1=c32,
                    scalar2=bq_t[:, j:j + 1], op0=MULT, op1=ADD)

            def alloc_pair(p):
                kT8 = ktp.tile([128, NT // 4, 512], F8, tag="kT",
                               name=f"kT{p}")
                # va2[p, he, t, 0:64] = V rows; [.., 64:80] = 1/16 so the
                # denominator rides along in the same DR matmul (M=80) and
                # its reciprocal directly carries the x16 outT prescale
                va2 = vaug.tile([128, 2, NT, 80], F8, tag="va", name=f"va{p}")
                nc.gpsimd.memset(va2[:, :, :, 64:80], 1.0 / 16.0)
                return kT8, va2

            def emit_k_chunk(p, kT8, n):
                # K^T chunk: [128(dh pair), 512 keys]
                pk = ps_acc.tile([128, 512], F32, tag="acc", name=f"pk{p}_{n}")
                for s in range(4):
                    for dd in range(ND // 2):
                        nc.tensor.matmul(
                            pk[:, 128 * s:128 * (s + 1)],
                            wk8[:, p, 2 * dd:2 * dd + 2, :],
                            xmov(dd, 4 * n + s),
                            start=(s == 0 and dd == 0),
                            stop=(s == 3 and dd == ND // 2 - 1),
                            perf_mode=DR, skip_group_check=True)
                nc.vector.tensor_scalar(
                    out=kT8[:, n, :], in0=pk, scalar1=c32,
                    scalar2=bk_t[:, p:p + 1], op0=MULT, op1=ADD)

            def emit_v_chunk(p, va2, n):
                # V chunk, token-major via stationary = x^T
                pv = ps_acc.tile([128, 512], F32, tag="acc", name=f"pv{p}_{n}")
                for s in range(4):
                    for dd in range(ND // 2):
                        nc.tensor.matmul(
                            pv[:, 128 * s:128 * (s + 1)],
                            xP8[:, 4 * n + s, 2 * dd:2 * dd + 2, :],
                            wv8[:, p, 2 * dd:2 * dd + 2, :],
                            start=(s == 0 and dd == 0),
                            stop=(s == 3 and dd == ND // 2 - 1),
                            perf_mode=DR, skip_group_check=True)
                nc.vector.tensor_scalar(
                    out=va2[:, :, 4 * n:4 * (n + 1), 0:64],
                    in0=pv.rearrange("p (t h x) -> p h t x", t=4, x=64),
                    scalar1=c32, scalar2=bv_t[:, p:p + 1], op0=MULT, op1=ADD)

            # =========================================================
            # Pipelined main loop over head pairs
            # =========================================================
            outT = bigp.tile([128, NTQ, ND, 128], F8, tag="outT", name="outT")

            po_tiles = {}
            emit_q(0)
            next_q = [1]
            cur = alloc_pair(0)
            # only the first K chunk up front; everything else is staggered
            # into pair 0's attention loop so the first exp starts ASAP
            emit_k_chunk(0, cur[0], 0)
            emit_v_chunk(0, cur[1], 0)

            for p in range(NPAIR):
                kT8, va2 = cur
                nxt = alloc_pair(p + 1) if p + 1 < NPAIR else None

                pav0 = ps_pav.tile([128, TQ], F32, tag="pav", name=f"pav0_{p}")
                pav1 = ps_pav.tile([128, TQ], F32, tag="pav", name=f"pav1_{p}")
                exq = {}
                for ch in range(NCHUNK):
                    for he in range(2):
                        ex = expp.tile([128, TPC, TQ], F8, tag="ex",
                                       name=f"ex{he}_{ch}")
                        exq[(he, ch)] = ex
                        for g in range(TPC // 2):
                            psc = ps_sc.tile([128, 2, TQ], F32, tag="sc",
                                             name="psc")
                            for s2 in range(2):
                                t = ch * TPC + 2 * g + s2
                                lhs = kT8[64 * he:64 * (he + 1),
                                          t // 4,
                                          128 * (t % 4):128 * (t % 4 + 1)]
                                rhs = qT8[64 * he:64 * (he + 1), p, :]
                                nc.tensor.matmul(psc[:, s2, :], lhs, rhs,
                                                 start=True, stop=True)
                            nc.scalar.activation(
                                out=ex[:, 2 * g:2 * g + 2, :], in_=psc,
                                func=mybir.ActivationFunctionType.Exp,
                                scale=SCALE)
                    # keep PE fed for the NEXT pair while ScalarE runs exp
                    while next_q[0] < min(ND, p + ch + 2):
                        emit_q(next_q[0])
                        next_q[0] += 1
                    if p == 0 and ch + 1 < NCHUNK:
                        emit_k_chunk(0, kT8, ch + 1)
                        emit_v_chunk(0, va2, ch + 1)
                    if nxt is not None:
                        emit_k_chunk(p + 1, nxt[0], ch)
                        emit_v_chunk(p + 1, nxt[1], ch)
                    for he in range(2):
                        ex = exq[(he, ch)]
                        pav = pav0 if he == 0 else pav1
                        for g in range(TPC // 2):
                            t2 = (ch * TPC) // 2 + g   # key tile-pair index
                            # [V | ones16] -> psum rows 0:80 (row 64 = denom)
                            nc.tensor.matmul(
                                pav[0:80, :], va2[:, he, 2 * t2:2 * t2 + 2, :],
                                ex[:, 2 * g:2 * g + 2, :],
                                start=(t2 == 0), stop=(t2 == NT // 2 - 1),
                                perf_mode=DR)

                if p == NPAIR - 1:
                    # start the out-projection's early partials (dd 0..2 use
                    # only pairs 0..5) before the rb matmuls below block the
                    # in-order PE queue behind this pair's DVE normalize
                    for i in range(2):
                        po = ps_sc.tile([128, 2, 512], F32, tag="sc",
                                        name=f"po{i}")
                        po_tiles[i] = po
                        for half in range(2):
                            for dd in range(ND // 2 - 1):
                                nc.tensor.matmul(
                                    po[:, half, :],
                                    outT[:, i, 2 * dd:2 * dd + 2, :],
                                    wo_t[:, 2 * dd:2 * dd + 2,
                                         512 * half:512 * (half + 1)],
                                    start=(dd == 0), stop=(dd == ND // 2 - 2),
                                    perf_mode=DR, skip_group_check=True)

                # ---- normalize: psum rows 64:80 of each pav hold 16
                # identical denominator copies (ridden at 1/16 scale, so
                # their reciprocal = 16/sum). Copy them to SBUF (plain
                # tensor_copy is offset-safe), fan out 16->64 partitions
                # with one SBUF->SBUF DMA (each source row read 4x via a
                # 0-step free dim), then reciprocal at partition base 0
                # (reciprocal_approx_fast corrupts at partition offsets).
                # No PE matmuls on this path.
                def fan4(src_ap):
                    a = [list(d) for d in src_ap.ap]
                    return bass.AP(tensor=src_ap.tensor, offset=src_ap.offset,
                                   ap=[a[0], [0, 4]] + a[1:])

                dnS = smallp.tile([128, 2, TQ], F32, tag="dns", name="dnS")
                nc.vector.tensor_copy(out=dnS[64:80, 0, :],
                                      in_=pav0[64:80, :])
                nc.vector.tensor_copy(out=dnS[64:80, 1, :],
                                      in_=pav1[64:80, :])
                dnF = smallp.tile([128, 2, TQ], F32, tag="rr1", name="dnF")
                nc.gpsimd.dma_start(out=dnF[0:64, 0, :],
                                    in_=fan4(dnS[64:80, 0, :]))
                nc.gpsimd.dma_start(out=dnF[0:64, 1, :],
                                    in_=fan4(dnS[64:80, 1, :]))
                rrec = smallp.tile([128, 2, TQ], F32, tag="rrec", name="rrec")
                nc.vector.reciprocal_approx_fast(out=rrec[0:64, 0, :],
                                                 in_=dnF[0:64, 0, :])
                nc.vector.reciprocal_approx_fast(out=rrec[0:64, 1, :],
                                                 in_=dnF[0:64, 1, :])
                nc.vector.tensor_mul(
                    out=outT[0:64, :, p, :],
                    in0=pav0[0:64, :].rearrange("p (a b) -> p a b", b=128),
                    in1=rrec[0:64, 0, :].rearrange("p (a b) -> p a b", b=128))
                tmp = smallp.tile([128, TQ], F8, tag="otmp", name="tmp")
                nc.vector.tensor_mul(out=tmp[0:64, :], in0=pav1[0:64, :],
                                     in1=rrec[0:64, 1, :])
                nc.gpsimd.dma_start(
                    out=outT[64:128, :, p, :],
                    in_=tmp[0:64, :].rearrange("p (a b) -> p a b", b=128))
                # precompute bo + x for the LayerNorm tail on the idle Pool
                if 2 <= p < 2 + NTQ:
                    nc.gpsimd.tensor_add(out=bxq[:, p - 2, :], in0=bo_b,
                                         in1=xq_t[:, p - 2, :])
                cur = nxt

            # =========================================================
            # Phase D: out-proj + residual + LayerNorm
            # =========================================================
            # prefetch the Sqrt act table while ScalarE is idle so the
            # first real sqrt in the LN chain doesn't eat the table load
            nc.scalar.activation(out=wsc, in_=eps_t,
                                 func=mybir.ActivationFunctionType.Sqrt,
                                 scale=1.0)
            # two-stage software pipeline: A(i) = out-proj finish + evac +
            # stats + sqrt + recip; B(i) = normalize + writeback. Emitting
            # A(i+1) before B(i) keeps the DVE queue busy during tile i's
            # sqrt round-trip (the in-order queue otherwise parks tile
            # i+1's evacuation behind tile i's normalize).
            def phase_a(i):
                if i in po_tiles:
                    po = po_tiles[i]
                    for half in range(2):
                        dd = ND // 2 - 1
                        nc.tensor.matmul(
                            po[:, half, :],
                            outT[:, i, 2 * dd:2 * dd + 2, :],
                            wo_t[:, 2 * dd:2 * dd + 2,
                                 512 * half:512 * (half + 1)],
                            start=False, stop=True,
                            perf_mode=DR, skip_group_check=True)
                else:
                    po = ps_sc.tile([128, 2, 512], F32, tag="sc", name="po")
                    for half in range(2):
                        dst = po[:, half, :]
                        for dd in range(ND // 2):
                            nc.tensor.matmul(
                                dst, outT[:, i, 2 * dd:2 * dd + 2, :],
                                wo_t[:, 2 * dd:2 * dd + 2,
                                     512 * half:512 * (half + 1)],
                                start=(dd == 0), stop=(dd == ND // 2 - 1),
                                perf_mode=DR)
                ysb = ybufp.tile([128, D], F32, tag="ysb")
                stats = statp.tile([128, 2, 6], F32, tag="stats")
                mv = statp.tile([128, 2], F32, tag="mv")
                yv = ysb.rearrange("p (a b) -> p a b", a=2)
                # per-half evac so bn_stats(half) starts as soon as that
                # half of the out-projection lands in PSUM
                for sg in range(2):
                    nc.vector.scalar_tensor_tensor(
                        out=yv[:, sg, :], in0=po[:, sg, :], scalar=1.0 / 256.0,
                        in1=bxq[:, i, 512 * sg:512 * (sg + 1)],
                        op0=MULT, op1=ADD)
                    nc.vector.bn_stats(out=stats[:, sg, :], in_=yv[:, sg, :])
                nc.vector.bn_aggr(out=mv, in_=stats)
                sd = statp.tile([128, 1], F32, tag="sd")
                nc.scalar.activation(out=sd, in_=mv[:, 1:2],
                                     func=mybir.ActivationFunctionType.Sqrt,
                                     bias=eps_t, scale=1.0)
                rstd = statp.tile([128, 1], F32, tag="rstd")
                nc.vector.reciprocal(out=rstd, in_=sd)
                return ysb, mv, rstd

            def phase_b(i, ysb, mv, rstd):
                # out-of-place normalize (in-place DVE/Pool tensor ops run
                # ~10-20x slower); per-half so the first half's writeback
                # overlaps the second half's normalize
                ysb2 = ybufp.tile([128, D], F32, tag="ysb2")
                for hf in range(2):
                    sl = slice(512 * hf, 512 * (hf + 1))
                    # half 1 on the (idle) Pool engine so both halves
                    # normalize in parallel instead of serializing on DVE
                    eng = nc.vector if (hf == 0 or apply_gb) else nc.gpsimd
                    eng.tensor_scalar(
                        out=ysb2[:, sl], in0=ysb[:, sl], scalar1=mv[:, 0:1],
                        scalar2=rstd, op0=mybir.AluOpType.subtract,
                        op1=MULT)
                    src = ysb2
                    if apply_gb:
                        nc.vector.tensor_mul(out=ysb[:, sl], in0=ysb2[:, sl],
                                             in1=gamma_b[:, sl])
                        nc.vector.tensor_add(out=ysb2[:, sl], in0=ysb[:, sl],
                                             in1=beta_b[:, sl])
                    q = nc.sync if hf == 0 else nc.scalar
                    q.dma_start(out=y_d[128 * i:128 * (i + 1), sl],
                                in_=src[:, sl])

            st = {0: phase_a(0), 1: phase_a(1)}
            phase_b(0, *st.pop(0))
            st[2] = phase_a(2)
            phase_b(1, *st.pop(1))
            st[3] = phase_a(3)
            phase_b(2, *st.pop(2))
            phase_b(3, *st.pop(3))

    nc.compile()
    return nc


_PROGRAM_CACHE = {}


def _get_program(apply_gb=True):
    key = ("v4", apply_gb)
    if key not in _PROGRAM_CACHE:
        _PROGRAM_CACHE[key] = _build_program(apply_gb)
    return _PROGRAM_CACHE[key]


def _pack_w8(w):
    # [p, otile, dtile, c] = 32*W[128*dtile+p, 128*otile+c], fp8
    w = (np.asarray(w, np.float32) * WSCALE).reshape(ND, 128, ND, 128)
    return np.ascontiguousarray(w.transpose(1, 2, 0, 3)).astype(NP_F8)


def _pack_wo(w):
    # [p, dtile, o] = 16*Wo[128*dtile+p, o], fp8
    w = (np.asarray(w, np.float32) * 16.0).reshape(ND, 128, D)
    return np.ascontiguousarray(w.transpose(1, 0, 2)).astype(NP_F8)


def _pack_b(b):
    # [p, otile] = b[128*otile+p]
    b = np.asarray(b, np.float32).reshape(ND, 128)
    return np.ascontiguousarray(b.transpose(1, 0))


OUT_SHAPE_PER_CORE = (TQ, D)


def _expected_shard(full_y, c):
    b = c // CORES_PER_BATCH
    off = TQ * (c % CORES_PER_BATCH)
    return full_y[b, off:off + TQ]


def _make_in_maps(x, Wq, bq, Wk, bk, Wv, bv, Wo, bo, gamma, beta):
    x = np.asarray(x, dtype=np.float32)
    wq_p, wk_p, wv_p = _pack_w8(Wq), _pack_w8(Wk), _pack_w8(Wv)
    wo_p = _pack_wo(Wo)
    b3_p = np.ascontiguousarray(
        np.stack([_pack_b(bq), _pack_b(bk), _pack_b(bv)], axis=1))
    in_maps = []
    for c in range(N_CORES):
        b = c // CORES_PER_BATCH
        off = TQ * (c % CORES_PER_BATCH)
        xb = np.concatenate([x[b, off:], x[b, :off]], axis=0)
        # [p, t, d, c] = x[128t+c, 128d+p]
        xbP8 = np.ascontiguousarray(
            xb.reshape(NT, 128, ND, 128).transpose(3, 0, 2, 1)).astype(NP_F8)
        in_maps.append({
            "xbP8": xbP8,
            "xq": np.ascontiguousarray(xb[0:TQ]),
            "wq8": wq_p, "wk8": wk_p, "wv8": wv_p, "wo": wo_p,
            "b3": b3_p,
            "bo": np.asarray(bo, np.float32),
            "gamma": np.asarray(gamma, np.float32),
            "beta": np.asarray(beta, np.float32),
        })
    return in_maps


def kernel(x, Wq, bq, Wk, bk, Wv, bv, Wo, bo, gamma, beta, _trace=False):
    apply_gb = not (np.allclose(np.asarray(gamma), 1.0)
                    and np.allclose(np.asarray(beta), 0.0))
    nc = _get_program(apply_gb)
    in_maps = _make_in_maps(x, Wq, bq, Wk, bk, Wv, bv, Wo, bo, gamma, beta)

    res = bass_utils.run_bass_kernel_spmd(
        nc, in_maps, list(range(N_CORES)), trace=_trace)

    y = np.empty((B, S, D), dtype=np.float32)
    for c in range(N_CORES):
        b = c // CORES_PER_BATCH
        off = TQ * (c % CORES_PER_BATCH)
        y[b, off:off + TQ] = res.results[c]["y"]

    kernel.last_exec_time_ns = res.exec_time_ns
    return y


kernel.last_exec_time_ns = None



# revision 58
# speedup vs baseline: 1.0909x; 1.0909x over previous
"""Trainium2 Bass kernel: MultiHeadAttention + residual + LayerNorm.

Problem shapes (hardcoded):
  x: (2, 2048, 1024) f32, 16 heads x 64 head_dim, scale = 64**-0.5
  y = LayerNorm(x + MHA(x))

Sharding: token-parallel over 8 cores. Core c handles batch b=c//4 and
query tokens [512*(c%4), 512*(c%4+1)) of that batch. Each core receives
its batch's full token sequence ROTATED so that its own 512 query tokens
are rows 0..511 (attention is permutation-invariant over keys). No
cross-core collectives.

Design (fp8 v5):
- fp8e4m3 operands for all matmuls; f32 residual/LayerNorm path.
  Weights host-prescaled x32 (their +-1/32 range lands in fp8
  subnormals otherwise), Wo x16; descales fold into PSUM evacuations.
- QKV projections, attn@V, and out-proj run in DoubleRow perf mode
  (2 contraction tiles per PE instruction). HW ISA constraints honored:
  stationary k-tile pairs contiguous, stationary free dim a multiple of
  16, DR outputs at partition base 0. Moving-operand pairs may stride.
- V is built token-major via stationary=x^T (a second, token-tile-major
  fp8 copy of x) -- no PE transposes; one fused (psum/32 + bv) -> fp8
  evacuation per 512 tokens.
- attn@V stationary is [V | (1/16)x16] (M=80): softmax denominators
  ride in psum rows 64:80 of the same accumulation at 1/16 scale.
  Normalization is PE-free: copy rows 64:80 to SBUF, fan out 16->64
  partitions with one SBUF-SBUF DMA (each source row read 4x via a
  0-step dim), reciprocal_approx_fast at partition base 0 (the custom
  DVE op corrupts at nonzero partition offsets), multiply. head1
  partition-shifts via one SBUF-SBUF DMA.
- Software pipelining: Q^T j-slices and the next pair's K/V chunks are
  emitted inside the previous pair's attention chunk loop, keeping the
  scalar engine (exp, the bottleneck: ~110us of irreducible work) fed
  across pair boundaries. PE warmup matmuls ramp the clock p-state and
  preload the Exp table during the initial DMAs; the Sqrt table is
  prefetched before the LayerNorm tail.
- DMA ordering: packed biases first, first-use weight/x slivers next,
  bulk weights after, Phase-D-only tensors (Wo, residual x, LN row
  constants) last. bo+x is pre-added on the Pool engine mid-kernel.
- LayerNorm tail: fused (po/256 + (bo+x)) evacuation, bn_stats/bn_aggr,
  gamma/beta skipped when they are identity (checked on host). The
  final normalize is OUT-OF-PLACE on DVE per 512-col half (in-place
  DVE/Pool tensor ops run 10-20x slower), each half's writeback on its
  own DMA queue.

(A token-sharded K/V AllGather variant -- compute K/V only for own 512
tokens, gather pairs across the batch's 4 cores -- was built and was
correct (see kernel_ag_299us.py) but lost ~25us to serial collective
latency plus SBUF-write contention during attention; the collective-
free version here is faster on this stack.)
"""

import sys

sys.path.insert(0, "/opt/trn_rl_repo")

import numpy as np
import ml_dtypes

import concourse.bass as bass
import concourse.bacc as bacc
import concourse.mybir as mybir
import concourse.tile as tile
from concourse import bass_utils

# ---- problem constants ----
B = 2
S = 2048
D = 1024
H = 16
DH = 64
SCALE = DH ** -0.5
EPS = 1e-5

N_CORES = 8
CORES_PER_BATCH = N_CORES // B
TQ = S // CORES_PER_BATCH          # 512 query tokens per core
NT = S // 128                      # 16 key tiles of 128
ND = D // 128                      # 8 dim tiles of 128
NPAIR = H // 2                     # 8 head pairs
NTQ = TQ // 128                    # 4 query tiles
WSCALE = 32.0                      # host premultiplies W by this
NCHUNK = 4                         # attention chunks per pair
TPC = NT // NCHUNK                 # key tiles per chunk

F32 = mybir.dt.float32
F32R = mybir.dt.float32r
BF16 = mybir.dt.bfloat16
F8 = mybir.dt.float8e4
DR = mybir.MatmulPerfMode.DoubleRow
MULT = mybir.AluOpType.mult
ADD = mybir.AluOpType.add

NP_F8 = ml_dtypes.float8_e4m3
NP_BF16 = ml_dtypes.bfloat16


def _build_program(apply_gb=True):
    nc = bacc.Bacc("TRN2", target_bir_lowering=False, debug=False,
                   num_devices=N_CORES)

    # ---- DRAM I/O ----
    xbP8_d = nc.dram_tensor("xbP8", (128, NT, ND, 128), F8,
                            kind="ExternalInput").ap()
    xq_d = nc.dram_tensor("xq", (TQ, D), F32, kind="ExternalInput").ap()
    wq8_d = nc.dram_tensor("wq8", (128, ND, ND, 128), F8,
                           kind="ExternalInput").ap()
    wk8_d = nc.dram_tensor("wk8", (128, ND, ND, 128), F8,
                           kind="ExternalInput").ap()
    wv8_d = nc.dram_tensor("wv8", (128, ND, ND, 128), F8,
                           kind="ExternalInput").ap()
    wo_d = nc.dram_tensor("wo", (128, ND, D), F8, kind="ExternalInput").ap()
    b3_d = nc.dram_tensor("b3", (128, 3, ND), F32, kind="ExternalInput").ap()
    bo_d = nc.dram_tensor("bo", (D,), F32, kind="ExternalInput").ap()
    gamma_d = nc.dram_tensor("gamma", (D,), F32, kind="ExternalInput").ap()
    beta_d = nc.dram_tensor("beta", (D,), F32, kind="ExternalInput").ap()
    y_d = nc.dram_tensor("y", (TQ, D), F32, kind="ExternalOutput").ap()

    def bcast_rows(src_row_ap, nrows):
        return bass.AP(tensor=src_row_ap.tensor, offset=src_row_ap.offset,
                       ap=[[0, nrows]] + [list(d) for d in src_row_ap.ap[-1:]])

    with tile.TileContext(nc) as tc:
        from contextlib import ExitStack
        with ExitStack() as ctx:
            consts = ctx.enter_context(tc.tile_pool(name="consts", bufs=1))
            bigp = ctx.enter_context(tc.tile_pool(name="big", bufs=1))
            ktp = ctx.enter_context(tc.tile_pool(name="ktp", bufs=3))
            vaug = ctx.enter_context(tc.tile_pool(name="vaug", bufs=3))
            expp = ctx.enter_context(tc.tile_pool(name="expp", bufs=8))
            smallp = ctx.enter_context(tc.tile_pool(name="small", bufs=2))
            ybufp = ctx.enter_context(tc.tile_pool(name="ybuf", bufs=4))
            statp = ctx.enter_context(tc.tile_pool(name="statp", bufs=4))

            ps_sc = ctx.enter_context(
                tc.tile_pool(name="ps_sc", bufs=2, space="PSUM"))
            ps_acc = ctx.enter_context(
                tc.tile_pool(name="ps_acc", bufs=2, space="PSUM"))
            ps_pav = ctx.enter_context(
                tc.tile_pool(name="ps_pav", bufs=2, space="PSUM"))

            # ---- tiny loads first: biases (one packed DMA), constants ----
            b3_t = consts.tile([128, 3, ND], F32)
            nc.sync.dma_start(out=b3_t, in_=b3_d)
            bq_t, bk_t, bv_t = b3_t[:, 0, :], b3_t[:, 1, :], b3_t[:, 2, :]
            eps_t = consts.tile([128, 1], F32)
            nc.vector.memset(eps_t, EPS)
            c32 = consts.tile([128, 1], F32)
            nc.vector.memset(c32, 1.0 / WSCALE)
            ones_f = consts.tile([128, 64], F32)
            nc.vector.memset(ones_f, 1.0 / 16.0)  # folds x16 into attn-out
            ones_r = consts.tile([128, 64], F32R)
            nc.vector.tensor_copy(out=ones_r, in_=ones_f)

            # ---- warmup: ramp the PE clock + preload the Exp table while
            # the first DMAs are in flight (both are off the data path) ----
            warm = ps_acc.tile([128, TQ], F32, tag="acc", name="warm")
            for w in range(18):
                nc.tensor.matmul(warm[0:64, 0:64], ones_r[0:1, :],
                                 ones_r[0:1, :], start=True, stop=True)
            wsc = smallp.tile([128, 1], F32, tag="wsc", name="wsc")
            nc.scalar.activation(out=wsc, in_=eps_t,
                                 func=mybir.ActivationFunctionType.Exp,
                                 scale=1.0)

            # ---- compute-critical loads: Wq, x (per 512-token chunk);
            # the first-use slivers go out on four queues in parallel (the
            # first QKV emission is gated by these ~900KB) ----
            wq8 = bigp.tile([128, ND, ND, 128], F8, tag="wq8", name="wq8")
            nc.sync.dma_start(out=wq8[:, 0, :, :], in_=wq8_d[:, 0, :, :])
            xP8 = bigp.tile([128, NT, ND, 128], F8, tag="xP8", name="xP8")
            nc.scalar.dma_start(out=xP8[:, 0:2], in_=xbP8_d[:, 0:2])
            nc.sync.dma_start(out=xP8[:, 2:4], in_=xbP8_d[:, 2:4])
            wk8 = bigp.tile([128, ND, ND, 128], F8, tag="wk8", name="wk8")
            nc.gpsimd.dma_start(out=wk8[:, 0, :, :], in_=wk8_d[:, 0, :, :])
            wv8 = bigp.tile([128, ND, ND, 128], F8, tag="wv8", name="wv8")
            nc.gpsimd.dma_start(out=wv8[:, 0, :, :], in_=wv8_d[:, 0, :, :])
            nc.sync.dma_start(out=wq8[:, 1:2], in_=wq8_d[:, 1:2])
            nc.sync.dma_start(out=xP8[:, 4:8], in_=xbP8_d[:, 4:8])
            nc.sync.dma_start(out=wk8[:, 1:2], in_=wk8_d[:, 1:2])
            nc.sync.dma_start(out=wv8[:, 1:2], in_=wv8_d[:, 1:2])
            nc.sync.dma_start(out=xP8[:, 8:12], in_=xbP8_d[:, 8:12])
            nc.sync.dma_start(out=wq8[:, 2:3], in_=wq8_d[:, 2:3])
            nc.sync.dma_start(out=xP8[:, 12:16], in_=xbP8_d[:, 12:16])
            nc.sync.dma_start(out=wq8[:, 3:ND], in_=wq8_d[:, 3:ND])
            nc.sync.dma_start(out=wk8[:, 2:ND], in_=wk8_d[:, 2:ND])
            nc.sync.dma_start(out=wv8[:, 2:ND], in_=wv8_d[:, 2:ND])

            # ---- Phase-D-only loads: Wo, residual x, LN row consts ----
            wo_t = bigp.tile([128, ND, D], F8, tag="wo", name="wo")
            nc.sync.dma_start(out=wo_t, in_=wo_d)
            xq_t = bigp.tile([128, NTQ, D], F32, tag="xq", name="xq")
            nc.sync.dma_start(
                out=xq_t, in_=xq_d.rearrange("(i p) d -> p i d", p=128))
            lnc = consts.tile([128, 3, D], F32, name="lnc")
            nc.sync.dma_start(out=lnc[:, 0, :], in_=bcast_rows(bo_d[None], 128))
            nc.sync.dma_start(out=lnc[:, 1, :],
                              in_=bcast_rows(gamma_d[None], 128))
            nc.sync.dma_start(out=lnc[:, 2, :],
                              in_=bcast_rows(beta_d[None], 128))
            bo_b, gamma_b, beta_b = lnc[:, 0, :], lnc[:, 1, :], lnc[:, 2, :]

            # ---- emission helpers (software pipelining) ----
            qT8 = bigp.tile([128, ND, TQ], F8, tag="qT8", name="qT8")
            bxq = bigp.tile([128, NTQ, D], F32, tag="bxq", name="bxq")

            def xmov(dd, t):
                # [128, 2(d-pair), 128 tokens] moving view of xP8 tile t
                return xP8[:, t, 2 * dd:2 * dd + 2, :]

            def emit_q(j):
                # qT8[p, j, tq] = q[tq, 128j+p]; q = (psum/32) + bq
                pq = ps_acc.tile([128, TQ], F32, tag="acc", name=f"pq{j}")
                for s in range(4):
                    for dd in range(ND // 2):
                        nc.tensor.matmul(
                            pq[:, 128 * s:128 * (s + 1)],
                            wq8[:, j, 2 * dd:2 * dd + 2, :], xmov(dd, s),
                            start=(s == 0 and dd == 0),
                            stop=(s == 3 and dd == ND // 2 - 1),
                            perf_mode=DR, skip_group_check=True)
                nc.vector.tensor_scalar(
                    out=qT8[:, j, :], in0=pq, scalar1=c32,
                    scalar2=bq_t[:, j:j + 1], op0=MULT, op1=ADD)

            def alloc_pair(p):
                kT8 = ktp.tile([128, NT // 4, 512], F8, tag="kT",
                               name=f"kT{p}")
                # va2[p, he, t, 0:64] = V rows; [.., 64:80] = 1/16 so the
                # denominator rides along in the same DR matmul (M=80) and
                # its reciprocal directly carries the x16 outT prescale
                va2 = vaug.tile([128, 2, NT, 80], F8, tag="va", name=f"va{p}")
                nc.gpsimd.memset(va2[:, :, :, 64:80], 1.0 / 16.0)
                return kT8, va2

            def emit_k_chunk(p, kT8, n):
                # K^T chunk: [128(dh pair), 512 keys]
                pk = ps_acc.tile([128, 512], F32, tag="acc", name=f"pk{p}_{n}")
                for s in range(4):
                    for dd in range(ND // 2):
                        nc.tensor.matmul(
                            pk[:, 128 * s:128 * (s + 1)],
                            wk8[:, p, 2 * dd:2 * dd + 2, :],
                            xmov(dd, 4 * n + s),
                            start=(s == 0 and dd == 0),
                            stop=(s == 3 and dd == ND // 2 - 1),
                            perf_mode=DR, skip_group_check=True)
                nc.vector.tensor_scalar(
                    out=kT8[:, n, :], in0=pk, scalar1=c32,
                    scalar2=bk_t[:, p:p + 1], op0=MULT, op1=ADD)

            def emit_v_chunk(p, va2, n):
                # V chunk, token-major via stationary = x^T
                pv = ps_acc.tile([128, 512], F32, tag="acc", name=f"pv{p}_{n}")
                for s in range(4):
                    for dd in range(ND // 2):
                        nc.tensor.matmul(
                            pv[:, 128 * s:128 * (s + 1)],
                            xP8[:, 4 * n + s, 2 * dd:2 * dd + 2, :],
                            wv8[:, p, 2 * dd:2 * dd + 2, :],
                            start=(s == 0 and dd == 0),
                            stop=(s == 3 and dd == ND // 2 - 1),
                            perf_mode=DR, skip_group_check=True)
                nc.vector.tensor_scalar(
                    out=va2[:, :, 4 * n:4 * (n + 1), 0:64],
                    in0=pv.rearrange("p (t h x) -> p h t x", t=4, x=64),
                    scalar1=c32, scalar2=bv_t[:, p:p + 1], op0=MULT, op1=ADD)

            # =========================================================
            # Pipelined main loop over head pairs
            # =========================================================
            outT = bigp.tile([128, NTQ, ND, 128], F8, tag="outT", name="outT")

            po_tiles = {}
            emit_q(0)
            next_q = [1]
            cur = alloc_pair(0)
            # only the first K chunk up front; everything else is staggered
            # into pair 0's attention loop so the first exp starts ASAP
            emit_k_chunk(0, cur[0], 0)
            emit_v_chunk(0, cur[1], 0)

            for p in range(NPAIR):
                kT8, va2 = cur
                nxt = alloc_pair(p + 1) if p + 1 < NPAIR else None

                pav0 = ps_pav.tile([128, TQ], F32, tag="pav", name=f"pav0_{p}")
                pav1 = ps_pav.tile([128, TQ], F32, tag="pav", name=f"pav1_{p}")
                exq = {}
                for ch in range(NCHUNK):
                    for he in range(2):
                        ex = expp.tile([128, TPC, TQ], F8, tag="ex",
                                       name=f"ex{he}_{ch}")
                        exq[(he, ch)] = ex
                        for g in range(TPC // 2):
                            psc = ps_sc.tile([128, 2, TQ], F32, tag="sc",
                                             name="psc")
                            for s2 in range(2):
                                t = ch * TPC + 2 * g + s2
                                lhs = kT8[64 * he:64 * (he + 1),
                                          t // 4,
                                          128 * (t % 4):128 * (t % 4 + 1)]
                                rhs = qT8[64 * he:64 * (he + 1), p, :]
                                nc.tensor.matmul(psc[:, s2, :], lhs, rhs,
                                                 start=True, stop=True)
                            nc.scalar.activation(
                                out=ex[:, 2 * g:2 * g + 2, :], in_=psc,
                                func=mybir.ActivationFunctionType.Exp,
                                scale=SCALE)
                    # keep PE fed for the NEXT pair while ScalarE runs exp
                    while next_q[0] < min(ND, p + ch + 2):
                        emit_q(next_q[0])
                        next_q[0] += 1
                    if p == 0 and ch + 1 < NCHUNK:
                        emit_k_chunk(0, kT8, ch + 1)
                        emit_v_chunk(0, va2, ch + 1)
                    if nxt is not None:
                        emit_k_chunk(p + 1, nxt[0], ch)
                        emit_v_chunk(p + 1, nxt[1], ch)
                    for he in range(2):
                        ex = exq[(he, ch)]
                        pav = pav0 if he == 0 else pav1
                        for g in range(TPC // 2):
                            t2 = (ch * TPC) // 2 + g   # key tile-pair index
                            # [V | ones16] -> psum rows 0:80 (row 64 = denom)
                            nc.tensor.matmul(
                                pav[0:80, :], va2[:, he, 2 * t2:2 * t2 + 2, :],
                                ex[:, 2 * g:2 * g + 2, :],
                                start=(t2 == 0), stop=(t2 == NT // 2 - 1),
                                perf_mode=DR)

                if p == NPAIR - 1:
                    # start the out-projection's early partials (dd 0..2 use
                    # only pairs 0..5) before the rb matmuls below block the
                    # in-order PE queue behind this pair's DVE normalize
                    for i in range(2):
                        po = ps_sc.tile([128, 2, 512], F32, tag="sc",
                                        name=f"po{i}")
                        po_tiles[i] = po
                        for half in range(2):
                            for dd in range(ND // 2 - 1):
                                nc.tensor.matmul(
                                    po[:, half, :],
                                    outT[:, i, 2 * dd:2 * dd + 2, :],
                                    wo_t[:, 2 * dd:2 * dd + 2,
                                         512 * half:512 * (half + 1)],
                                    start=(dd == 0), stop=(dd == ND // 2 - 2),
                                    perf_mode=DR, skip_group_check=True)

                # ---- normalize: psum rows 64:80 of each pav hold 16
                # identical denominator copies (ridden at 1/16 scale, so
                # their reciprocal = 16/sum). Copy them to SBUF (plain
                # tensor_copy is offset-safe), fan out 16->64 partitions
                # with one SBUF->SBUF DMA (each source row read 4x via a
                # 0-step free dim), then reciprocal at partition base 0
                # (reciprocal_approx_fast corrupts at partition offsets).
                # No PE matmuls on this path.
                def fan4(src_ap):
                    a = [list(d) for d in src_ap.ap]
                    return bass.AP(tensor=src_ap.tensor, offset=src_ap.offset,
                                   ap=[a[0], [0, 4]] + a[1:])

                dnS = smallp.tile([128, 2, TQ], F32, tag="dns", name="dnS")
                nc.vector.tensor_copy(out=dnS[64:80, 0, :],
                                      in_=pav0[64:80, :])
                nc.vector.tensor_copy(out=dnS[64:80, 1, :],
                                      in_=pav1[64:80, :])
                dnF = smallp.tile([128, 2, TQ], F32, tag="rr1", name="dnF")
                nc.gpsimd.dma_start(out=dnF[0:64, 0, :],
                                    in_=fan4(dnS[64:80, 0, :]))
                nc.gpsimd.dma_start(out=dnF[0:64, 1, :],
                                    in_=fan4(dnS[64:80, 1, :]))
                rrec = smallp.tile([128, 2, TQ], F32, tag="rrec", name="rrec")
                nc.vector.reciprocal_approx_fast(out=rrec[0:64, 0, :],
                                                 in_=dnF[0:64, 0, :])
                nc.vector.reciprocal_approx_fast(out=rrec[0:64, 1, :],
                                                 in_=dnF[0:64, 1, :])
                nc.vector.tensor_mul(
                    out=outT[0:64, :, p, :],
                    in0=pav0[0:64, :].rearrange("p (a b) -> p a b", b=128),
                    in1=rrec[0:64, 0, :].rearrange("p (a b) -> p a b", b=128))
                tmp = smallp.tile([128, TQ], F8, tag="otmp", name="tmp")
                nc.vector.tensor_mul(out=tmp[0:64, :], in0=pav1[0:64, :],
                                     in1=rrec[0:64, 1, :])
                nc.gpsimd.dma_start(
                    out=outT[64:128, :, p, :],
                    in_=tmp[0:64, :].rearrange("p (a b) -> p a b", b=128))
                # precompute bo + x for the LayerNorm tail on the idle Pool
                if 2 <= p < 2 + NTQ:
                    nc.gpsimd.tensor_add(out=bxq[:, p - 2, :], in0=bo_b,
                                         in1=xq_t[:, p - 2, :])
                cur = nxt

            # =========================================================
            # Phase D: out-proj + residual + LayerNorm
            # =========================================================
            # prefetch the Sqrt act table while ScalarE is idle so the
            # first real sqrt in the LN chain doesn't eat the table load
            nc.scalar.activation(out=wsc, in_=eps_t,
                                 func=mybir.ActivationFunctionType.Sqrt,
                                 scale=1.0)
            # two-stage software pipeline: A(i) = out-proj finish + evac +
            # stats + sqrt + recip; B(i) = normalize + writeback. Emitting
            # A(i+1) before B(i) keeps the DVE queue busy during tile i's
            # sqrt round-trip (the in-order queue otherwise parks tile
            # i+1's evacuation behind tile i's normalize).
            def phase_a(i):
                if i in po_tiles:
                    po = po_tiles[i]
                    for half in range(2):
                        dd = ND // 2 - 1
                        nc.tensor.matmul(
                            po[:, half, :],
                            outT[:, i, 2 * dd:2 * dd + 2, :],
                            wo_t[:, 2 * dd:2 * dd + 2,
                                 512 * half:512 * (half + 1)],
                            start=False, stop=True,
                            perf_mode=DR, skip_group_check=True)
                else:
                    po = ps_sc.tile([128, 2, 512], F32, tag="sc", name="po")
                    for half in range(2):
                        dst = po[:, half, :]
                        for dd in range(ND // 2):
                            nc.tensor.matmul(
                                dst, outT[:, i, 2 * dd:2 * dd + 2, :],
                                wo_t[:, 2 * dd:2 * dd + 2,
                                     512 * half:512 * (half + 1)],
                                start=(dd == 0), stop=(dd == ND // 2 - 1),
                                perf_mode=DR)
                ysb = ybufp.tile([128, D], F32, tag="ysb")
                stats = statp.tile([128, 2, 6], F32, tag="stats")
                mv = statp.tile([128, 2], F32, tag="mv")
                yv = ysb.rearrange("p (a b) -> p a b", a=2)
                # per-half evac so bn_stats(half) starts as soon as that
                # half of the out-projection lands in PSUM
                for sg in range(2):
                    nc.vector.scalar_tensor_tensor(
                        out=yv[:, sg, :], in0=po[:, sg, :], scalar=1.0 / 256.0,
                        in1=bxq[:, i, 512 * sg:512 * (sg + 1)],
                        op0=MULT, op1=ADD)
                    nc.vector.bn_stats(out=stats[:, sg, :], in_=yv[:, sg, :])
                nc.vector.bn_aggr(out=mv, in_=stats)
                sd = statp.tile([128, 1], F32, tag="sd")
                nc.scalar.activation(out=sd, in_=mv[:, 1:2],
                                     func=mybir.ActivationFunctionType.Sqrt,
                                     bias=eps_t, scale=1.0)
                rstd = statp.tile([128, 1], F32, tag="rstd")
                nc.vector.reciprocal(out=rstd, in_=sd)
                return ysb, mv, rstd

            def phase_b(i, ysb, mv, rstd):
                # out-of-place normalize (in-place DVE/Pool tensor ops run
                # ~10-20x slower); per-half so the first half's writeback
                # overlaps the second half's normalize
                ysb2 = ybufp.tile([128, D], F32, tag="ysb2")
                for hf in range(2):
                    sl = slice(512 * hf, 512 * (hf + 1))
                    nc.vector.tensor_scalar(
                        out=ysb2[:, sl], in0=ysb[:, sl], scalar1=mv[:, 0:1],
                        scalar2=rstd, op0=mybir.AluOpType.subtract,
                        op1=MULT)
                    src = ysb2
                    if apply_gb:
                        nc.vector.tensor_mul(out=ysb[:, sl], in0=ysb2[:, sl],
                                             in1=gamma_b[:, sl])
                        nc.vector.tensor_add(out=ysb2[:, sl], in0=ysb[:, sl],
                                             in1=beta_b[:, sl])
                    q = nc.sync if hf == 0 else nc.scalar
                    q.dma_start(out=y_d[128 * i:128 * (i + 1), sl],
                                in_=src[:, sl])

            st = {0: phase_a(0), 1: phase_a(1)}
            phase_b(0, *st.pop(0))
            st[2] = phase_a(2)
            phase_b(1, *st.pop(1))
            st[3] = phase_a(3)
            phase_b(2, *st.pop(2))
            phase_b(3, *st.pop(3))

    nc.compile()
    return nc


_PROGRAM_CACHE = {}


def _get_program(apply_gb=True):
    key = ("v4", apply_gb)
    if key not in _PROGRAM_CACHE:
        _PROGRAM_CACHE[key] = _build_program(apply_gb)
    return _PROGRAM_CACHE[key]


def _pack_w8(w):
    # [p, otile, dtile, c] = 32*W[128*dtile+p, 128*otile+c], fp8
    w = (np.asarray(w, np.float32) * WSCALE).reshape(ND, 128, ND, 128)
    return np.ascontiguousarray(w.transpose(1, 2, 0, 3)).astype(NP_F8)


def _pack_wo(w):
    # [p, dtile, o] = 16*Wo[128*dtile+p, o], fp8
    w = (np.asarray(w, np.float32) * 16.0).reshape(ND, 128, D)
    return np.ascontiguousarray(w.transpose(1, 0, 2)).astype(NP_F8)


def _pack_b(b):
    # [p, otile] = b[128*otile+p]
    b = np.asarray(b, np.float32).reshape(ND, 128)
    return np.ascontiguousarray(b.transpose(1, 0))


OUT_SHAPE_PER_CORE = (TQ, D)


def _expected_shard(full_y, c):
    b = c // CORES_PER_BATCH
    off = TQ * (c % CORES_PER_BATCH)
    return full_y[b, off:off + TQ]


def _make_in_maps(x, Wq, bq, Wk, bk, Wv, bv, Wo, bo, gamma, beta):
    x = np.asarray(x, dtype=np.float32)
    wq_p, wk_p, wv_p = _pack_w8(Wq), _pack_w8(Wk), _pack_w8(Wv)
    wo_p = _pack_wo(Wo)
    b3_p = np.ascontiguousarray(
        np.stack([_pack_b(bq), _pack_b(bk), _pack_b(bv)], axis=1))
    in_maps = []
    for c in range(N_CORES):
        b = c // CORES_PER_BATCH
        off = TQ * (c % CORES_PER_BATCH)
        xb = np.concatenate([x[b, off:], x[b, :off]], axis=0)
        # [p, t, d, c] = x[128t+c, 128d+p]
        xbP8 = np.ascontiguousarray(
            xb.reshape(NT, 128, ND, 128).transpose(3, 0, 2, 1)).astype(NP_F8)
        in_maps.append({
            "xbP8": xbP8,
            "xq": np.ascontiguousarray(xb[0:TQ]),
            "wq8": wq_p, "wk8": wk_p, "wv8": wv_p, "wo": wo_p,
            "b3": b3_p,
            "bo": np.asarray(bo, np.float32),
            "gamma": np.asarray(gamma, np.float32),
            "beta": np.asarray(beta, np.float32),
        })
    return in_maps


def kernel(x, Wq, bq, Wk, bk, Wv, bv, Wo, bo, gamma, beta, _trace=False):
    apply_gb = not (np.allclose(np.asarray(gamma), 1.0)
                    and np.allclose(np.asarray(beta), 0.0))
    nc = _get_program(apply_gb)
    in_maps = _make_in_maps(x, Wq, bq, Wk, bk, Wv, bv, Wo, bo, gamma, beta)

    res = bass_utils.run_bass_kernel_spmd(
        nc, in_maps, list(range(N_CORES)), trace=_trace)

    y = np.empty((B, S, D), dtype=np.float32)
    for c in range(N_CORES):
        b = c // CORES_PER_BATCH
        off = TQ * (c % CORES_PER_BATCH)
        y[b, off:off + TQ] = res.results[c]["y"]

    kernel.last_exec_time_ns = res.exec_time_ns
    return y


kernel.last_exec_time_ns = None

